# revision 11
# baseline (speedup 1.0000x reference)
"""NetVLAD Trainium2 Bass kernel (v5).

Full inputs -> full output. Shards batch N=64 across 8 NeuronCores
(8 samples per core), runs one SPMD Bass/Tile kernel, gathers.

v5 over v4 (trace-driven; v4 was DMA-bound at ~88% with ~305GB/s):
  - xt ships BF16 instead of f32 and tail-packed (896px main block +
    4px tail DMA) instead of padded-to-1024: xt bytes drop 2.3x.
    Precision sim'd: harness rel err ~9.6e-3 vs 2e-2 gate.
  - out ships bf16 (host upcasts).
  - ssq via scalar_tensor_tensor(x,x,accum_out) - one fused op per
    chunk, no bn_stats combine. Chunks split DVE/ACT/GPSIMD (gpsimd
    was idle in v4).
  - aT broadcast-mul moved to GPSIMD.
  - normc = reciprocal(invn) on DVE (drops the +0.5 exp on ACT);
    exact 1/invn keeps the S-term cancellation.
  - out = main - S*cen folded into mm2's PSUM accumulation as a
    diag(S) matmul vs cen_neg (USE_DIAG), killing the ACT scale +
    DVE add; one bf16 PSUM->SBUF copy remains.
  - chunk 7 of xt holds stale SBUF rows 4:128; all NaN/garbage paths
    are confined to those partitions and excluded from mm2 by pw=4.
"""
import numpy as np

N, C, H, W = 64, 512, 30, 30
P = H * W              # 900
PPAD = 1024            # logsb padded pixel count (pad logits = 0)
K = 64
NCORES = 8
S = N // NCORES        # samples per core
CCH = 4                # channel chunks of 128
PCHUNKS = 8            # pixel chunks per sample: 7 full + 4-row tail
PFULL = 7
PTAIL = P - 128 * PFULL  # 4

# ssq chunk assignment: engine per pixel-chunk index
# (gpsimd lacks the TensorScalarPtr opcode - Pool can't do fused
# square+accum, so ssq splits across DVE and ACT only)
SSQ_DVE = (0, 1, 2, 3, 4)
SSQ_ACT = (5, 6, 7)
SSQ_GPS = ()
USE_DIAG = True

_cache = {}


def _build_module(repeat=1):
    import concourse.bacc as bacc
    import concourse.bass as bass
    import concourse.tile as tile
    import concourse.mybir as mybir

    F32 = mybir.dt.float32
    F32R = mybir.dt.float32r
    F16 = mybir.dt.float16
    BF16 = mybir.dt.bfloat16
    AF = mybir.ActivationFunctionType
    AX = mybir.AxisListType
    ALU = mybir.AluOpType

    nc = bacc.Bacc("TRN2", target_bir_lowering=False, debug=False,
                   num_devices=NCORES)

    # Pin the activation functions we use to the one table set containing
    # them all (the load-insertion pass maps each activation to the FIRST
    # set listing its function, which otherwise thrashes).
    import concourse.hw_specs as hw_specs
    _tabs = hw_specs.get_activation_tables(nc.m.arch)
    _target = "natural_log_exp_and_others"
    _orig_get_tables = bacc.get_activation_tables
    if _target in _tabs:
        _pin = {AF.Ln, AF.Exp, AF.Copy, AF.Identity, AF.Square}
        _patched = {
            name: (set(funcs) | _pin if name == _target else set(funcs) - _pin)
            for name, funcs in _tabs.items()
        }
        bacc.get_activation_tables = lambda arch: _patched

    F8 = mybir.dt.float8e4
    x_d = nc.dram_tensor("x", [S, 128, CCH, P], F8, kind="ExternalInput").ap()
    xt_d = nc.dram_tensor("xt", [S, 128, PFULL, C], BF16, kind="ExternalInput").ap()
    xt4_d = nc.dram_tensor("xt4", [S, PTAIL, C], BF16, kind="ExternalInput").ap()
    cwT_d = nc.dram_tensor("cwT", [C, K], BF16, kind="ExternalInput").ap()
    cen_d = nc.dram_tensor("cenneg", [K, C], F32R, kind="ExternalInput").ap()
    id_d = nc.dram_tensor("ident", [128, 128], BF16, kind="ExternalInput").ap()
    out_d = nc.dram_tensor("vlad", [S, K, C], BF16, kind="ExternalOutput").ap()

    with tile.TileContext(nc) as tc:
        with (
            tc.tile_pool(name="consts", bufs=1) as consts,
            tc.tile_pool(name="xnat", bufs=5) as xnat_pool,
            tc.tile_pool(name="xtp", bufs=5) as xt_pool,
            tc.tile_pool(name="work", bufs=5) as work,
            tc.tile_pool(name="sqscr", bufs=2) as sqscr_pool,
            tc.tile_pool(name="outsb", bufs=4) as outsb_pool,
            tc.tile_pool(name="pvec", bufs=6) as pvec_pool,
            tc.tile_pool(name="pslogits", bufs=1, space="PSUM") as pslogits,
            tc.tile_pool(name="pslogT", bufs=2, space="PSUM") as pslogT,
            tc.tile_pool(name="psmain", bufs=2, space="PSUM") as psmain,
            tc.tile_pool(name="psS", bufs=2, space="PSUM") as psS,
        ):
            # ---- constants ----
            cwT = consts.tile([128, CCH, K], BF16, tag="cwT")
            nc.sync.dma_start(
                cwT[:], cwT_d.rearrange("(j i) k -> i j k", i=128))
            ident = consts.tile([128, 128], BF16, tag="ident")
            nc.sync.dma_start(ident[:], id_d)
            cen = consts.tile([K, C], F32R, tag="cen")
            nc.sync.dma_start(cen[:], cen_d)

            ND = len(SSQ_DVE)

            def ssq_act(scr, ssqc, xt, pj):
                nc.scalar.activation(
                    scr[:], xt[:, pj, :], AF.Square,
                    accum_out=ssqc[:, pj:pj + 1])

            def stage1a(s):
                """DMAs + mm1 (PE work first; no DVE/ACT head-of-line)."""
                xna = xnat_pool.tile([128, CCH, P], F8, tag="xna")
                nc.sync.dma_start(xna[:], x_d[s])
                xt = xt_pool.tile([128, PCHUNKS, C], BF16, tag="xt")
                nc.sync.dma_start(xt[:, 0:PFULL, :], xt_d[s])
                nc.sync.dma_start(xt[0:PTAIL, PFULL, :], xt4_d[s])

                # mm1: logits[K, P] (4 stationary cwT chunks)
                logA = pslogits.tile([K, 450], F32, tag="logA")
                logB = pslogits.tile([K, 450], F32, tag="logB")
                for j in range(CCH):
                    nc.tensor.matmul(
                        logA[:], cwT[:, j, :], xna[:, j, 0:450],
                        start=(j == 0), stop=(j == CCH - 1))
                    nc.tensor.matmul(
                        logB[:], cwT[:, j, :], xna[:, j, 450:900],
                        start=(j == 0), stop=(j == CCH - 1))
                return s, xt, logA, logB

            def stage1b(st):
                """Logit copies, ssq, transposes (emitted late so older
                samples' ready ops sit at each engine queue's head)."""
                s, xt, logA, logB = st
                ssqc = pvec_pool.tile([128, PCHUNKS], F32, tag="ssqc")
                stats = sqscr_pool.tile([128, ND, 6], F32, tag="stats")
                for pj in SSQ_DVE:
                    nc.vector.bn_stats(stats[:, pj, :], xt[:, pj, :])
                # logits -> sbuf bf16 (DVE+ACT halves); pads pre-zeroed
                logsb = work.tile([K, PPAD], BF16, tag="logsb")
                nc.vector.tensor_copy(logsb[:, 0:450], logA[:])
                nc.scalar.copy(logsb[:, 450:900], logB[:])
                scr_a = sqscr_pool.tile([128, C], BF16, tag="scra")
                for pj in SSQ_ACT:
                    ssq_act(scr_a, ssqc, xt, pj)
                # combine bn_stats -> ssq on GPSIMD (emitted after 2b's
                # tcol/aT so it cannot block them in the Pool FIFO):
                # ssq = cvar_e + cvar_o + 256*(mean_e^2 + mean_o^2)
                sqm = pvec_pool.tile([128, ND, 2], F32, tag="sqm")
                nc.gpsimd.tensor_mul(
                    sqm[:], stats[:, :, 1:6:3], stats[:, :, 1:6:3])
                vsum = pvec_pool.tile([128, ND], F32, tag="vsum")
                nc.gpsimd.tensor_add(
                    vsum[:], stats[:, :, 2:3], stats[:, :, 5:6])
                msum = pvec_pool.tile([128, ND], F32, tag="msum")
                nc.gpsimd.tensor_add(msum[:], sqm[:, :, 0], sqm[:, :, 1])
                m256 = pvec_pool.tile([128, ND], F32, tag="m256")
                nc.gpsimd.tensor_scalar(
                    m256[:], msum[:], 256.0, None,
                    op0=mybir.AluOpType.mult)
                nc.gpsimd.tensor_add(ssqc[:, 0:ND], vsum[:], m256[:])

                # transpose logits -> logT [pixel, K] (shared identity)
                logT = pslogT.tile([128, PCHUNKS * K], BF16, tag="logT")
                for pj in range(PCHUNKS):
                    nc.tensor.matmul(
                        logT[:, K * pj:K * (pj + 1)],
                        logsb[:, 128 * pj:128 * (pj + 1)],
                        ident[0:K, 0:K],
                        is_transpose=True,
                        skip_group_check=True,
                    )
                return s, xt, ssqc, logT

            def stage2a(st):
                """ln / exp on ACT, prescale on DVE."""
                s, xt, ssqc, logT = st
                lssq = pvec_pool.tile([128, PCHUNKS], F32, tag="lssq")
                nc.scalar.activation(lssq[:], ssqc[:], AF.Ln)
                # rv = [invn | scol]: one reciprocal later covers both
                rv = pvec_pool.tile([128, 16], F32, tag="rv")
                invn = rv[:, 0:PCHUNKS]
                nc.scalar.activation(invn, lssq[:], AF.Exp, scale=-0.5)
                lsc = work.tile([128, PCHUNKS, K], F32, tag="lsc")
                nc.vector.tensor_mul(
                    lsc[:],
                    logT[:].rearrange("i (c k) -> i c k", k=K),
                    invn.to_broadcast([128, PCHUNKS, K]))
                return s, xt, rv, lsc

            def stage2b(st):
                """exp, softmax sums, aT."""
                s, xt, rv, lsc = st
                e_sb = work.tile([128, PCHUNKS * K], F32, tag="esb")
                nc.scalar.activation(e_sb[:], lsc[:], AF.Exp)

                nc.vector.reduce_sum(
                    rv[:, 8:16], e_sb[:].rearrange("i (c k) -> i c k", k=K),
                    axis=AX.X)
                # one reciprocal: rvi = [normc=1/invn | invs=1/scol] fp16
                rvi = pvec_pool.tile([128, 16], F16, tag="rvi")
                with nc.allow_low_precision(reason="norm/invs fit fp16"):
                    nc.vector.reciprocal(rvi[:], rv[:])
                tcol = pvec_pool.tile([128, PCHUNKS], F32, tag="tcol")
                nc.gpsimd.tensor_mul(tcol[:], rvi[:, 8:16], rv[:, 0:PCHUNKS])

                # aT = e * t (gpsimd broadcast mul, fp16 out for mm2)
                aT = work.tile([128, PCHUNKS, K], F16, tag="aT")
                nc.gpsimd.tensor_mul(
                    aT[:],
                    e_sb[:].rearrange("i (c k) -> i c k", k=K),
                    tcol[:].to_broadcast([128, PCHUNKS, K]))
                return s, xt, aT, rvi

            def stage3(st):
                s, xt, aT, rvi = st
                # mm2: main += aT.T @ xt ; S += aT.T @ norm (shared lhsT)
                main_ps = psmain.tile([K, C], F32, tag="main")
                S_ps = psS.tile([K, 2], F32, tag="Sps")
                for pj in range(PCHUNKS):
                    pw = PTAIL if pj == PCHUNKS - 1 else 128
                    last = pj == PCHUNKS - 1
                    nc.tensor.matmul(
                        main_ps[:], aT[0:pw, pj, :],
                        xt[0:pw, pj, :],
                        start=(pj == 0), stop=(last and not USE_DIAG))
                    nc.tensor.matmul(
                        S_ps[:], aT[0:pw, pj, :],
                        rvi[0:pw, pj:pj + 2],
                        start=(pj == 0), stop=last)

                if USE_DIAG:
                    # diag(S) @ cen_neg accumulated into main_ps
                    diag = pvec_pool.tile([K, K], F32R, tag="diag")
                    nc.vector.tensor_mul(
                        diag[:], ident[0:K, 0:K],
                        S_ps[:, 0:1].to_broadcast([K, K]))
                    nc.tensor.matmul(
                        main_ps[:], diag[:], cen[:],
                        start=False, stop=True)
                    out_sb = outsb_pool.tile([K, C], BF16, tag="outsb")
                    nc.scalar.copy(out_sb[:], main_ps[:])
                else:
                    tmp = outsb_pool.tile([K, C], F32, tag="tmp")
                    nc.scalar.activation(
                        tmp[:], cen[:].bitcast(F32), AF.Copy,
                        scale=S_ps[:, 0:1])
                    out_sb = outsb_pool.tile([K, C], BF16, tag="outsb")
                    nc.vector.tensor_add(out_sb[:], main_ps[:], tmp[:])
                nc.sync.dma_start(out_d[s], out_sb[:])

            # pre-zero the logsb pad columns in all ring buffers
            for _ in range(5):
                lb0 = work.tile([K, PPAD], BF16, tag="logsb")
                nc.vector.memset(lb0[:, P:PPAD], 0.0)

            # skewed software pipeline (3 samples deep), emission order
            # chosen so each engine FIFO sees ready work first:
            # iter k emits: 3(k-2), 1a(k), 2a(k-1), 2b(k-1), 1b(k)
            samples = [s for _ in range(repeat) for s in range(S)]
            prev1 = prev2 = None
            for s in samples:
                if prev2 is not None:
                    stage3(prev2)
                a = stage1a(s)
                s2 = None
                if prev1 is not None:
                    mid = stage2a(prev1)
                    s2 = stage2b(mid)
                b = stage1b(a)
                prev2 = s2
                prev1 = b
            last = stage2b(stage2a(prev1))
            stage3(prev2)
            stage3(last)

    try:
        nc.compile()
    finally:
        bacc.get_activation_tables = _orig_get_tables
    return nc


def _get_nc(repeat=1):
    key = ("nc", repeat)
    if key not in _cache:
        _cache[key] = _build_module(repeat)
    return _cache[key]


def build_in_maps(x, conv_w, centroids):
    import ml_dtypes

    bf16 = ml_dtypes.bfloat16
    x = np.asarray(x, dtype=np.float32)
    conv_w = np.asarray(conv_w, dtype=np.float32)
    centroids = np.asarray(centroids, dtype=np.float32)

    import concourse.mybir as mybir
    fp8 = mybir.dt.np(mybir.dt.float8e4)
    xs = x.reshape(N, C, P)
    # natural layout reordered to the SBUF tile shape [N, 128, CCH, P]
    x_bf = np.ascontiguousarray(
        xs.astype(fp8).reshape(N, CCH, 128, P).transpose(0, 2, 1, 3))
    # pixel-major bf16: 7 full 128-px chunks + 4-px tail
    xt_f = np.ascontiguousarray(xs.transpose(0, 2, 1).astype(bf16))  # [N,P,C]
    xt_main = np.ascontiguousarray(
        xt_f[:, :128 * PFULL].reshape(N, PFULL, 128, C).transpose(0, 2, 1, 3))
    xt_tail = np.ascontiguousarray(xt_f[:, 128 * PFULL:])  # [N, 4, C]
    cwT = np.ascontiguousarray(conv_w.T).astype(bf16)   # [C, K]
    ident = np.eye(128, dtype=bf16)
    cen_neg = np.ascontiguousarray(-centroids)

    in_maps = []
    for core in range(NCORES):
        sl = slice(core * S, (core + 1) * S)
        in_maps.append({
            "x": np.ascontiguousarray(x_bf[sl]),
            "xt": np.ascontiguousarray(xt_main[sl]),
            "xt4": np.ascontiguousarray(xt_tail[sl]),
            "cwT": cwT, "cenneg": cen_neg, "ident": ident,
        })
    return in_maps


def kernel(x, conv_w, centroids):
    from concourse.bass_utils import run_bass_kernel_spmd

    nc = _get_nc()
    in_maps = build_in_maps(x, conv_w, centroids)
    res = run_bass_kernel_spmd(nc, in_maps, core_ids=list(range(NCORES)))
    out = np.concatenate([r["vlad"] for r in res.results], axis=0)
    return out.reshape(N, K, C).astype(np.float32)
